# revision 7
# baseline (speedup 1.0000x reference)
"""Trainium2 Bass kernel for nn_AttentionRGCN (2-layer RGCN, 100k nodes,
1.6M edges, H=256), sharded across 8 NeuronCores.

Strategy (graph/data parallel, per sharding hint):
- Nodes are partitioned 8 ways type-uniformly (6250 compound + 3750 target
  + 2500 disease per core, padded to 6272/3840/2560 -> 12672 rows/core,
  99 dst tiles of 128).
- Each core projects its own nodes, then an AllGather replicates the full
  [101376, 256] hidden table h_full into every core's DRAM per layer.
- Each core owns the edges whose dst lands in its node range.  Edges are
  grouped into (dst-tile, relation) cells; host balancing permutes nodes
  within each (core, type) segment so each cell holds <= B_CELL*128 edges.
- Per 128-edge batch: indirect-DMA row gather from h_full, a one-hot
  selection matrix (dst-slot == iota) built on DVE, and a PE matmul
  sel^T @ msg accumulating the per-relation aggregate in PSUM.  The
  aggregate is transposed on PE and pushed through the relation weight,
  accumulating root + all relations + bias (+residual) in PSUM, then
  LayerNorm -> next layer.  Final Wo matmul per tile.
- All per-core variation (gather indices, dst slots, edge padding) is
  data; the SPMD program is identical on all 8 cores.
"""

import numpy as np

# ---------------------------------------------------------------- constants
N_C, N_T, N_D = 50000, 30000, 20000
N = N_C + N_T + N_D
R = 4
E_R = 400000
L = 2
H = 256
EPS = 1e-5
NCORES = 8

SEG_REAL = (N_C // NCORES, N_T // NCORES, N_D // NCORES)   # 6250, 3750, 2500
SEG_PAD = (6272, 3840, 2560)                               # 128-multiples
S_CORE = sum(SEG_PAD)                                      # 12672 rows/core
TILES = S_CORE // 128                                      # 99
N_FULL = S_CORE * NCORES                                   # 101376
IN_DIMS = (256, 512, 128)
K_AUG = (384, 640, 256)    # aug(+1 ones row) padded up to 128-multiples

_CACHE = {}


# ---------------------------------------------------------------- host prep
def _balance_tiles(deg, seg_pad, cap):
    """Permute node order inside one (core, type) segment so that every
    (128-node tile, relation) cell has degree sum <= cap.  deg: [n_real, R].
    Returns positions into segment rows, or None if cap infeasible."""
    n_real = deg.shape[0]
    ntiles = seg_pad // 128
    loads = np.zeros((ntiles, R), np.int64)
    counts = np.zeros(ntiles, np.int64)
    order = np.argsort(-deg.max(axis=1), kind="stable")
    tile_of = np.empty(n_real, np.int32)
    for nd in order:
        nl = loads + deg[nd][None, :]            # [ntiles, R]
        feas = (counts < 128) & (nl <= cap).all(axis=1)
        if not feas.any():
            return None
        m = nl.max(axis=1)
        m[~feas] = 1 << 60
        best = int(np.argmin(m))
        tile_of[nd] = best
        loads[best] += deg[nd]
        counts[best] += 1
    pos = np.empty(n_real, np.int64)
    nxt = (np.arange(ntiles) * 128).astype(np.int64)
    for nd in range(n_real):
        t = tile_of[nd]
        pos[nd] = nxt[t]
        nxt[t] += 1
    return pos


def _host_prep(inputs):
    rng = np.random.default_rng(0)
    edge_index = np.asarray(inputs["edge_index"])
    src_g = edge_index[0].astype(np.int64)
    dst_g = edge_index[1].astype(np.int64)
    rel = np.repeat(np.arange(R), E_R)

    # ---- node -> (core, segment row) assignment with per-core balancing
    # type/global-id helpers
    def type_of(g):
        return np.where(g < N_C, 0, np.where(g < N_C + N_T, 1, 2))

    # per-node degree as dst, per relation (same edges both layers)
    deg = np.zeros((N, R), np.int64)
    np.add.at(deg, (dst_g, rel), 1)

    seg_base = (0, SEG_PAD[0], SEG_PAD[0] + SEG_PAD[1])
    type_base_g = (0, N_C, N_C + N_T)

    # position of global node g inside h_full
    hpos = np.empty(N, np.int64)
    cap = None
    for cap_try in (512, 640, 768):
        ok = True
        hpos_try = np.empty(N, np.int64)
        for c in range(NCORES):
            for ty in range(3):
                n_real = SEG_REAL[ty]
                g0 = type_base_g[ty] + c * n_real
                gids = np.arange(g0, g0 + n_real)
                pos = _balance_tiles(deg[gids], SEG_PAD[ty], cap_try)
                if pos is None:
                    ok = False
                    break
                hpos_try[gids] = c * S_CORE + seg_base[ty] + pos
            if not ok:
                break
        if ok:
            # verify achieved cap per (tile, r) cell globally
            hp_dst = hpos_try[dst_g]
            tile_idx = hp_dst // 128
            cell = tile_idx * R + rel
            cnt = np.bincount(cell, minlength=(N_FULL // 128) * R)
            if cnt.max() <= cap_try:
                hpos = hpos_try
                cap = cap_try
                break
    assert cap is not None, "tile balancing failed"
    b_cell = cap // 128

    # ---- per-core edge batches
    hp_src = hpos[src_g]
    hp_dst = hpos[dst_g]
    owner = hp_dst // S_CORE
    loc = hp_dst % S_CORE
    tile_loc = loc // 128
    slot = loc % 128

    n_batch = TILES * R * b_cell
    idx_all = np.zeros((NCORES, 128, n_batch), np.int32)
    slot_all = np.full((NCORES, 128, n_batch), 255.0, np.float32)
    for c in range(NCORES):
        m = owner == c
        e_t = tile_loc[m].astype(np.int64)
        e_r = rel[m].astype(np.int64)
        e_s = slot[m]
        e_src = hp_src[m]
        cell_id = e_t * R + e_r
        ordr = np.argsort(cell_id, kind="stable")
        cell_id = cell_id[ordr]
        e_s = e_s[ordr]
        e_src = e_src[ordr]
        # within-cell rank
        cell_start = np.searchsorted(cell_id, np.arange(TILES * R))
        counts = np.diff(np.append(cell_start, len(cell_id)))
        assert counts.max() <= cap
        rank = np.arange(len(cell_id)) - cell_start[cell_id]
        # batch column = cell*b_cell + rank//128 ; partition = rank%128
        col = cell_id * b_cell + rank // 128
        par = rank % 128
        idx_all[c, par, col] = e_src
        slot_all[c, par, col] = e_s
        # pad slots: point at a valid row (row 0 of table); slot stays 255
    return {
        "hpos": hpos, "b_cell": int(b_cell), "n_batch": int(n_batch),
        "idx_all": idx_all, "slot_all": slot_all,
    }


def _proj_inputs(inputs, hpos):
    """Per-core transposed+augmented projection inputs and weights."""
    xs = [np.asarray(inputs["x_compound"]), np.asarray(inputs["x_target"]),
          np.asarray(inputs["x_disease"])]
    Ws = [np.asarray(inputs["Wp_compound"]), np.asarray(inputs["Wp_target"]),
          np.asarray(inputs["Wp_disease"])]
    bs = [np.asarray(inputs["bp_compound"]), np.asarray(inputs["bp_target"]),
          np.asarray(inputs["bp_disease"])]
    type_base_g = (0, N_C, N_C + N_T)
    seg_base = (0, SEG_PAD[0], SEG_PAD[0] + SEG_PAD[1])
    xT_cores = []   # per type: [NCORES, K_AUG, SEG_PAD]
    for ty in range(3):
        n_real, n_pad, kaug, ind = (SEG_REAL[ty], SEG_PAD[ty], K_AUG[ty],
                                    IN_DIMS[ty])
        per_core = np.zeros((NCORES, kaug, n_pad), np.float32)
        for c in range(NCORES):
            g0 = type_base_g[ty] + c * n_real
            gids = np.arange(g0, g0 + n_real)
            pos = (hpos[gids] - c * S_CORE - seg_base[ty]).astype(np.int64)
            blk = np.zeros((kaug, n_pad), np.float32)
            blk[:ind, pos] = xs[ty][gids - type_base_g[ty]].T
            blk[ind, pos] = 1.0   # ones row for bias
            per_core[c] = blk
        xT_cores.append(per_core)
    W_aug = []
    for ty in range(3):
        kaug, ind = K_AUG[ty], IN_DIMS[ty]
        w = np.zeros((kaug, H), np.float32)
        w[:ind] = Ws[ty]
        w[ind] = bs[ty]
        W_aug.append(w)
    return xT_cores, W_aug


# ------------------------------------------------------------- device build
def _build_program(b_cell, n_batch):
    import concourse.bass as bass
    import concourse.bacc as bacc
    import concourse.tile as tile
    from concourse import mybir
    from concourse.masks import make_identity

    f32 = mybir.dt.float32
    nc = bacc.Bacc("TRN2", target_bir_lowering=False, debug=False,
                   num_devices=NCORES)

    # ---- dram tensors (identical across cores; contents differ)
    t_in = {}
    for ty, nm in enumerate(("xc", "xt", "xd")):
        t_in[nm] = nc.dram_tensor(nm, [K_AUG[ty], SEG_PAD[ty]], f32,
                                  kind="ExternalInput").ap()
        t_in["W" + nm] = nc.dram_tensor("W" + nm, [K_AUG[ty], H], f32,
                                        kind="ExternalInput").ap()
    t_in["idx"] = nc.dram_tensor("idx", [128, n_batch], mybir.dt.int32,
                                 kind="ExternalInput").ap()
    t_in["dslot"] = nc.dram_tensor("dslot", [128, n_batch], f32,
                                   kind="ExternalInput").ap()
    t_in["iota"] = nc.dram_tensor("iota", [128, 128], f32,
                                  kind="ExternalInput").ap()
    t_in["rgcn_w"] = nc.dram_tensor("rgcn_w", [L * R * H, H], f32,
                                    kind="ExternalInput").ap()
    t_in["rgcn_root"] = nc.dram_tensor("rgcn_root", [L * H, H], f32,
                                       kind="ExternalInput").ap()
    t_in["rgcn_b"] = nc.dram_tensor("rgcn_b", [L, H], f32,
                                    kind="ExternalInput").ap()
    t_in["ln_g"] = nc.dram_tensor("ln_g", [L, H], f32,
                                  kind="ExternalInput").ap()
    t_in["ln_b"] = nc.dram_tensor("ln_b", [L, H], f32,
                                  kind="ExternalInput").ap()
    t_in["Wo"] = nc.dram_tensor("Wo", [H, H], f32, kind="ExternalInput").ap()
    t_in["bo"] = nc.dram_tensor("bo", [1, H], f32, kind="ExternalInput").ap()
    y_out = nc.dram_tensor("y", [S_CORE, H], f32, kind="ExternalOutput").ap()

    def bcast_row(dram_ap_row):
        """AP reading one DRAM row [1, H] broadcast into 128 partitions."""
        a = dram_ap_row
        return bass.AP(tensor=a.tensor, offset=a.offset,
                       ap=[[0, 128]] + list(a.ap[1:]))

    with tile.TileContext(nc) as tc:
        with tc.tile_pool(name="const", bufs=1) as constp, \
             tc.tile_pool(name="wp", bufs=1) as wp, \
             tc.tile_pool(name="proj", bufs=3) as projp, \
             tc.tile_pool(name="msg", bufs=10) as msgp, \
             tc.tile_pool(name="sel", bufs=8) as selp, \
             tc.tile_pool(name="work", bufs=4) as workp, \
             tc.tile_pool(name="hown", bufs=3) as hop, \
             tc.tile_pool(name="psA", bufs=2, space="PSUM") as psA, \
             tc.tile_pool(name="psB", bufs=2, space="PSUM") as psB, \
             tc.tile_pool(name="psT", bufs=2, space="PSUM") as psT, \
             tc.tile_pool(name="dram", bufs=1, space="DRAM") as dram:

            ident = constp.tile([128, 128], f32)
            make_identity(nc, ident[:])
            iota_t = constp.tile([128, 128], f32)
            nc.sync.dma_start(out=iota_t[:], in_=t_in["iota"][:])
            idx_t = constp.tile([128, n_batch], mybir.dt.int32)
            nc.sync.dma_start(out=idx_t[:], in_=t_in["idx"][:])
            dslot_t = constp.tile([128, n_batch], f32)
            nc.sync.dma_start(out=dslot_t[:], in_=t_in["dslot"][:])
            eps_t = constp.tile([128, 1], f32)
            nc.vector.memset(eps_t[:], EPS)

            # weights to SBUF
            w_proj = []
            for ty, nm in enumerate(("xc", "xt", "xd")):
                kt = K_AUG[ty] // 128
                w = wp.tile([128, kt * H], f32, tag=f"w{nm}")
                nc.sync.dma_start(
                    out=w[:].rearrange("p (k h) -> p k h", h=H),
                    in_=t_in["W" + nm].rearrange("(k p) h -> p k h", p=128))
                w_proj.append(w)
            w_rel = wp.tile([128, L * R * 2 * H], f32)   # [l,r,khalf] blocks
            nc.sync.dma_start(
                out=w_rel[:].rearrange("p (b h) -> p b h", h=H),
                in_=t_in["rgcn_w"].rearrange("(b p) h -> p b h", p=128))
            w_root = wp.tile([128, L * 2 * H], f32)
            nc.sync.dma_start(
                out=w_root[:].rearrange("p (b h) -> p b h", h=H),
                in_=t_in["rgcn_root"].rearrange("(b p) h -> p b h", p=128))
            w_o = wp.tile([128, 2 * H], f32)
            nc.sync.dma_start(
                out=w_o[:].rearrange("p (b h) -> p b h", h=H),
                in_=t_in["Wo"].rearrange("(b p) h -> p b h", p=128))
            bias_l = []
            for l in range(L):
                bt = wp.tile([128, H], f32, tag=f"rb{l}")
                nc.sync.dma_start(out=bt[:],
                                  in_=bcast_row(t_in["rgcn_b"][l:l + 1, :]))
                gt = wp.tile([128, H], f32, tag=f"lg{l}")
                nc.sync.dma_start(out=gt[:],
                                  in_=bcast_row(t_in["ln_g"][l:l + 1, :]))
                lt = wp.tile([128, H], f32, tag=f"lb{l}")
                nc.sync.dma_start(out=lt[:],
                                  in_=bcast_row(t_in["ln_b"][l:l + 1, :]))
                bias_l.append((bt, gt, lt))
            bo_t = wp.tile([128, H], f32)
            nc.sync.dma_start(out=bo_t[:], in_=bcast_row(t_in["bo"][0:1, :]))

            # dram buffers
            ag_in = [dram.tile([S_CORE, H], f32, tag=f"agin{l}",
                               name=f"agin{l}") for l in range(L)]
            h_full = [dram.tile([N_FULL, H], f32, addr_space="Shared",
                                tag=f"hf{l}", name=f"hf{l}")
                      for l in range(L)]

            # ---------------- projections -> ag_in[0]
            seg_row = (0, SEG_PAD[0], SEG_PAD[0] + SEG_PAD[1])
            for ty, nm in enumerate(("xc", "xt", "xd")):
                kt = K_AUG[ty] // 128
                ntile = SEG_PAD[ty] // 128
                for rt in range(ntile):
                    lhs = projp.tile([128, kt * 128], f32, tag="lhs")
                    nc.sync.dma_start(
                        out=lhs[:].rearrange("p (k c) -> p k c", c=128),
                        in_=t_in[nm][:, rt * 128:(rt + 1) * 128].rearrange(
                            "(k p) c -> p k c", p=128))
                    ps = psA.tile([128, H], f32, space="PSUM", tag="out")
                    for k in range(kt):
                        nc.tensor.matmul(
                            out=ps[:],
                            lhsT=lhs[:, k * 128:(k + 1) * 128],
                            rhs=w_proj[ty][:, k * H:(k + 1) * H],
                            start=(k == 0), stop=(k == kt - 1))
                    ht = projp.tile([128, H], f32, tag="hproj")
                    nc.vector.tensor_copy(out=ht[:], in_=ps[:])
                    row0 = seg_row[ty] + rt * 128
                    nc.sync.dma_start(out=ag_in[0][row0:row0 + 128, :],
                                      in_=ht[:])

            # ---------------- layers
            for l in range(L):
                nc.gpsimd.collective_compute(
                    "AllGather", mybir.AluOpType.bypass,
                    replica_groups=[list(range(NCORES))],
                    ins=[ag_in[l].opt()], outs=[h_full[l].opt()],
                )
                hf = h_full[l]
                for t in range(TILES):
                    h_own = hop.tile([128, H], f32, tag="hown")
                    nc.sync.dma_start(out=h_own[:],
                                      in_=ag_in[l][t * 128:(t + 1) * 128, :])
                    # root: transpose h_own then 2 matmuls
                    out_ps = psA.tile([128, H], f32, space="PSUM", tag="out")
                    hT = []
                    for half in range(2):
                        tp = psT.tile([128, 128], f32, space="PSUM", tag="tp")
                        nc.tensor.transpose(
                            out=tp[:],
                            in_=h_own[:, half * 128:(half + 1) * 128],
                            identity=ident[:])
                        hTs = workp.tile([128, 128], f32, tag="hT")
                        nc.scalar.copy(out=hTs[:], in_=tp[:])
                        hT.append(hTs)
                    for half in range(2):
                        nc.tensor.matmul(
                            out=out_ps[:], lhsT=hT[half][:],
                            rhs=w_root[:, (l * 2 + half) * H:
                                       (l * 2 + half + 1) * H],
                            start=(half == 0), stop=False)
                    # relations
                    for r in range(R):
                        agg_ps = psB.tile([128, H], f32, space="PSUM",
                                          tag="agg")
                        for j in range(b_cell):
                            b = (t * R + r) * b_cell + j
                            msg = msgp.tile([128, H], f32, tag="msg")
                            nc.gpsimd.indirect_dma_start(
                                out=msg[:], out_offset=None, in_=hf[:],
                                in_offset=bass.IndirectOffsetOnAxis(
                                    ap=idx_t[:, b:b + 1], axis=0))
                            sel = selp.tile([128, 128], f32, tag="sel")
                            nc.vector.tensor_tensor(
                                out=sel[:],
                                in0=dslot_t[:, b:b + 1].to_broadcast(
                                    [128, 128]),
                                in1=iota_t[:],
                                op=mybir.AluOpType.is_equal)
                            nc.tensor.matmul(out=agg_ps[:], lhsT=sel[:],
                                             rhs=msg[:], start=(j == 0),
                                             stop=(j == b_cell - 1))
                        agg_sb = workp.tile([128, H], f32, tag="aggsb")
                        nc.vector.tensor_copy(out=agg_sb[:], in_=agg_ps[:])
                        for half in range(2):
                            tp = psT.tile([128, 128], f32, space="PSUM",
                                          tag="tp")
                            nc.tensor.transpose(
                                out=tp[:],
                                in_=agg_sb[:, half * 128:(half + 1) * 128],
                                identity=ident[:])
                            aT = workp.tile([128, 128], f32, tag="aT")
                            nc.scalar.copy(out=aT[:], in_=tp[:])
                            wblk = ((l * R + r) * 2 + half) * H
                            nc.tensor.matmul(
                                out=out_ps[:], lhsT=aT[:],
                                rhs=w_rel[:, wblk:wblk + H],
                                start=False,
                                stop=(r == R - 1 and half == 1))
                    # bias + residual + LN
                    bt, gt, lbt = bias_l[l]
                    x = workp.tile([128, H], f32, tag="x")
                    nc.vector.tensor_add(out=x[:], in0=out_ps[:], in1=bt[:])
                    if l > 0:
                        nc.vector.tensor_add(out=x[:], in0=x[:],
                                             in1=h_own[:])
                    stats = workp.tile([128, 6], f32, tag="st")
                    nc.vector.bn_stats(out=stats[:], in_=x[:])
                    mv = workp.tile([128, 2], f32, tag="mv")
                    nc.vector.bn_aggr(out=mv[:], in_=stats[:])
                    rstd = workp.tile([128, 1], f32, tag="rstd")
                    nc.scalar.activation(
                        out=rstd[:], in_=mv[:, 1:2],
                        func=mybir.ActivationFunctionType.Sqrt,
                        bias=eps_t[:], scale=1.0)
                    nc.vector.reciprocal(out=rstd[:], in_=rstd[:])
                    nc.vector.tensor_scalar(
                        out=x[:], in0=x[:], scalar1=mv[:, 0:1],
                        scalar2=rstd[:],
                        op0=mybir.AluOpType.subtract,
                        op1=mybir.AluOpType.mult)
                    nc.vector.tensor_mul(out=x[:], in0=x[:], in1=gt[:])
                    nc.vector.tensor_add(out=x[:], in0=x[:], in1=lbt[:])
                    if l < L - 1:
                        nc.sync.dma_start(
                            out=ag_in[l + 1][t * 128:(t + 1) * 128, :],
                            in_=x[:])
                    else:
                        # final y = x @ Wo + bo
                        fps = psA.tile([128, H], f32, space="PSUM",
                                       tag="out")
                        xT = []
                        for half in range(2):
                            tp = psT.tile([128, 128], f32, space="PSUM",
                                          tag="tp")
                            nc.tensor.transpose(
                                out=tp[:],
                                in_=x[:, half * 128:(half + 1) * 128],
                                identity=ident[:])
                            xts = workp.tile([128, 128], f32, tag="xT")
                            nc.scalar.copy(out=xts[:], in_=tp[:])
                            xT.append(xts)
                        for half in range(2):
                            nc.tensor.matmul(
                                out=fps[:], lhsT=xT[half][:],
                                rhs=w_o[:, half * H:(half + 1) * H],
                                start=(half == 0), stop=(half == 1))
                        yt = workp.tile([128, H], f32, tag="yt")
                        nc.vector.tensor_add(out=yt[:], in0=fps[:],
                                             in1=bo_t[:])
                        nc.sync.dma_start(
                            out=y_out[t * 128:(t + 1) * 128, :], in_=yt[:])

    nc.compile()
    return nc


# ------------------------------------------------------------------- kernel
def kernel(**inputs):
    from concourse.bass_utils import run_bass_kernel_spmd

    prep = _host_prep(inputs)
    b_cell, n_batch = prep["b_cell"], prep["n_batch"]
    key = (b_cell, n_batch)
    if key not in _CACHE:
        _CACHE[key] = _build_program(b_cell, n_batch)
    nc = _CACHE[key]

    xT_cores, W_aug = _proj_inputs(inputs, prep["hpos"])
    rgcn_w = np.asarray(inputs["rgcn_w"], np.float32).reshape(L * R * H, H)
    rgcn_root = np.asarray(inputs["rgcn_root"], np.float32).reshape(L * H, H)
    iota = np.tile(np.arange(128, dtype=np.float32)[None, :], (128, 1))

    in_maps = []
    for c in range(NCORES):
        in_maps.append({
            "xc": xT_cores[0][c], "xt": xT_cores[1][c], "xd": xT_cores[2][c],
            "Wxc": W_aug[0], "Wxt": W_aug[1], "Wxd": W_aug[2],
            "idx": prep["idx_all"][c], "dslot": prep["slot_all"][c],
            "iota": iota,
            "rgcn_w": rgcn_w, "rgcn_root": rgcn_root,
            "rgcn_b": np.asarray(inputs["rgcn_b"], np.float32),
            "ln_g": np.asarray(inputs["ln_g"], np.float32),
            "ln_b": np.asarray(inputs["ln_b"], np.float32),
            "Wo": np.asarray(inputs["Wo"], np.float32),
            "bo": np.asarray(inputs["bo"], np.float32).reshape(1, H),
        })

    res = run_bass_kernel_spmd(nc, in_maps, core_ids=list(range(NCORES)),
                               trace=getattr(kernel, "_TRACE", False))
    kernel.last_results = res

    # reassemble: y_full[hpos[g]] = core outputs concatenated
    y_all = np.concatenate([res.results[c]["y"] for c in range(NCORES)],
                           axis=0)  # [N_FULL, H]
    y_full = y_all[prep["hpos"]]
    return (y_full[:N_C], y_full[N_C:N_C + N_T], y_full[N_C + N_T:])
